# revision 21
# baseline (speedup 1.0000x reference)
"""Trainium2 Bass kernel for: Conv3d(3->16, k=3x3x3, VALID) + bias -> min over
depth -> softmax over channels.

Input  x: (16, 3, 32, 128, 128) f32   [N, C_in, D, H, W]
Weight w: (16, 3, 3, 3, 3) f32        [C_out, C_in, kD, kH, kW]
Bias   b: (16,) f32
Output  : (16, 16, 126, 126) f32      [N, C_out, H_out, W_out]

Data-parallel over batch: 2 batches per core x 8 cores. Two phases per
core, so the strict-FIFO engine queues in the hot loop never carry an
instruction whose input depends on another queue's backlog (that feedback
is what stalled earlier variants):

Phase 1 (hot loop; PE/ACT/DVE/GpSimd fully decoupled):
  - x stored per (batch, h-half) as one [128, 8768] bf16 tile: strip r
    (partition quadrant 32r) holds 30 rows = (10 input depths 8r..8r+9) x
    (ci 3); free dim = local (h, w) flattened (66 or 64 h-rows + pad).
  - Conv as 4 row-packed matmuls per (chunk, khw): tile r = [K<=30, M=128,
    N=512] at tile_position (32r, 0); M = 8 local douts x 16 co; 9
    accumulating MMs over (kh,kw) with free-dim-shifted rhs. Weight col
    block[(dl,ci), (dll,co)] = w[co,ci,dl-dll,kh,kw]. PSUM supertile
    [128, 4*512]: bank r = strip r's 8 douts; 2-deep ping-pong owns all 8
    banks and nothing else ever allocates PSUM in this phase.
  - Strip 3 douts 30,31 don't exist: their weight cols are 0 except a
    BIG=32768 entry at khw=0 against a constant-1.0 rhs row, so those
    psum lanes hold +32768 and never win the depth-min.
  - ACT's only job: copy each chunk's psum to SBUF bf16 (frees the bank).
  - DVE's only job: fold the 4 banks with two contiguous bf16 mins
    (pair-merged across chunks to amortize op overhead), plus the
    128->64->32->16 partition tree-min per 8-chunk group (DMA shift + TT
    min, stages spread 2 chunks apart on a tick schedule).
  - GpSimd's only job: quad memsets + tree-shift DMAs + the per-group
    regroup DMA into a persistent collall buffer
    [128 = 8co+j, 8 groups * 512] (out partition p=8co+j <- walk co,j,s).

Phase 2 (after the loop, PSUM free, ~15us, two pipelined 4-group halves):
  - ACT exp with fused bias (min(y)+b == min(y+b)) -> bf16.
  - co-sums via PE ones-matmul (lhsT[128,8] bf16, p%8==j) into psum.
  - DVE reciprocal_approx_fast -> bf16.
  - broadcast via PE ones_bc-matmul (rec[j] -> all p with p%8==j).
  - DVE multiply (psum f32 x exp bf16) and one output DMA per group
    (y DRAM h/w-padded to [NB, 16, 128, 128]; host slices [:126, :126]).
"""

import os
import sys

sys.path.insert(0, "/opt/trn_rl_repo")

import numpy as np
import ml_dtypes

import concourse.bass as bass
import concourse.bacc as bacc
import concourse.tile as tile
import concourse.mybir as mybir
import concourse.bass_isa as bass_isa
from concourse import bass_utils

F32 = mybir.dt.float32
BF16 = mybir.dt.bfloat16

N_CORES = 8
NB = 2           # batches per core
CI = 3
D = 32
H = 128
W = 128
CO = 16
CHUNK = 512
HOUT = 126
WOUT = 126
PAD = 320
QF = 66 * W + PAD  # quad tile free size (worst case hh=0)
BIG = 32768.0
NG = 8           # 8-chunk groups per core (4 items x 2)

_COMPILED = {}


def _strip_depths(r):
    return 10 if r < 3 else 8


def _kr(r, khw):
    if r < 3:
        return 30
    return 25 if khw == 0 else 24


def _build_weight_blocks(conv_weight):
    """[128, 9*128]: strip r rows 32r+(3*dl+ci); col khw*128 + dll*16 + co
    = w[co, ci, dl-dll, kh, kw] (0 outside kd range / dout>=30). Row 120
    (strip 3 local 24) carries BIG at khw=0 for the dout 30/31 lanes."""
    wb = np.zeros((128, 9 * 128), dtype=np.float32)
    for r in range(4):
        for dl in range(_strip_depths(r)):
            for ci in range(CI):
                row = 32 * r + 3 * dl + ci
                for khw in range(9):
                    kh, kw = khw // 3, khw % 3
                    for dll in range(8):
                        kd = dl - dll
                        if 8 * r + dll < 30 and 0 <= kd <= 2:
                            wb[row, khw * 128 + dll * 16:
                               khw * 128 + dll * 16 + 16] = \
                                conv_weight[:, ci, kd, kh, kw]
    for dll in (6, 7):
        wb[120, dll * 16:dll * 16 + 16] = BIG
    return wb.astype(ml_dtypes.bfloat16)


def _build_ones():
    """[128, 8] bf16: col j sums partitions {8*co + j : co}."""
    ones = np.zeros((128, 8), dtype=np.float32)
    for p in range(128):
        ones[p, p % 8] = 1.0
    return ones.astype(ml_dtypes.bfloat16)


def _build_ones_bc():
    """[8, 128] f32: row j broadcasts over its stride-8 partition set
    (f32 to match the f32 reciprocal as matmul moving operand)."""
    return np.ascontiguousarray(
        _build_ones().T.astype(np.float32))


def _build_bias128(conv_bias):
    """[128, 1]: partition 8*co + j -> bias[co]."""
    b = np.zeros((128, 1), dtype=np.float32)
    for p in range(128):
        b[p, 0] = conv_bias[p // 8]
    return b


def _emit_kernel(tc):
    nc = tc.nc
    x_ap = nc.dram_tensor("x", [NB, D, CI, H, W], BF16,
                          kind="ExternalInput").ap()
    w_ap = nc.dram_tensor("w", [128, 9 * 128], BF16,
                          kind="ExternalInput").ap()
    bias_ap = nc.dram_tensor("bias", [128, 1], F32, kind="ExternalInput").ap()
    ones_ap = nc.dram_tensor("ones", [128, 8], BF16,
                             kind="ExternalInput").ap()
    ones_bc_ap = nc.dram_tensor("ones_bc", [8, 128], F32,
                                kind="ExternalInput").ap()
    row1_ap = nc.dram_tensor("row1", [1, 66 * W], BF16,
                             kind="ExternalInput").ap()
    # h/w-padded output; host slices [:, :, :126, :126]
    y_ap = nc.dram_tensor("y", [NB, CO, 128, 128], F32,
                          kind="ExternalOutput").ap()

    from contextlib import ExitStack

    with ExitStack() as ctx:
        const_pool = ctx.enter_context(tc.tile_pool(name="const", bufs=1))
        in_pool = ctx.enter_context(tc.tile_pool(name="in", bufs=2))
        m4w_pool = ctx.enter_context(tc.tile_pool(name="m4w", bufs=3))
        ev_pool = ctx.enter_context(tc.tile_pool(name="ev", bufs=4))
        sm_pool = ctx.enter_context(tc.tile_pool(name="sm", bufs=2))
        psum_pool = ctx.enter_context(tc.tile_pool(name="ps", bufs=2,
                                                   space="PSUM"))

        w_sb = const_pool.tile([128, 9 * 128], BF16, tag="w")
        bias_sb = const_pool.tile([128, 1], F32, tag="bias")
        ones_sb = const_pool.tile([128, 8], BF16, tag="ones")
        ones_bc_sb = const_pool.tile([8, 128], F32, tag="onesbc")
        collall = const_pool.tile([128, NG * CHUNK], BF16, tag="collall")

        def load_consts():
            nc.sync.dma_start(w_sb[:, :], w_ap[:, :])
            nc.sync.dma_start(bias_sb[:, :], bias_ap[:, :])
            nc.sync.dma_start(ones_sb[:, :], ones_ap[:, :])
            nc.sync.dma_start(ones_bc_sb[:, :], ones_bc_ap[:, :])

        MIN = mybir.AluOpType.min

        # ---- tick-scheduled per-group tree stages ---------------------
        actions = {}

        def sched(tick, fn):
            actions.setdefault(tick, []).append(fn)

        def run_tick(tick):
            for fn in actions.pop(tick, []):
                fn()

        def sched_group_tree(t0, st):
            """128->16 partition tree-min for group st (last chunk at tick
            t0), one stage every 2 chunks; the result lands in collall."""
            m4w_t, g = st["m4w"], st["g"]

            def s_sh0():
                sh = sm_pool.tile([64, 8 * CHUNK], BF16, tag="sh")
                st["sh"] = sh
                nc.gpsimd.dma_start(sh[0:64, :], m4w_t[64:128, :])

            def s_l1():
                nc.vector.tensor_tensor(
                    out=m4w_t[0:64, :], in0=m4w_t[0:64, :],
                    in1=st["sh"][0:64, :], op=MIN)
                nc.gpsimd.dma_start(st["sh"][0:32, :], m4w_t[32:64, :])

            def s_l2():
                nc.vector.tensor_tensor(
                    out=m4w_t[0:32, :], in0=m4w_t[0:32, :],
                    in1=st["sh"][0:32, :], op=MIN)
                nc.gpsimd.dma_start(st["sh"][0:16, :], m4w_t[16:32, :])

            def s_l3():
                nc.vector.tensor_tensor(
                    out=m4w_t[0:16, :], in0=m4w_t[0:16, :],
                    in1=st["sh"][0:16, :], op=MIN)
                # regroup mins: out partition p = 8*co + j <- walk (co,j,s)
                nc.gpsimd.dma_start(
                    collall[:, g * CHUNK:(g + 1) * CHUNK],
                    m4w_t[0:16, :].rearrange("co (j s) -> co j s", j=8))

            if g < NG - 1:
                for i, fn in enumerate((s_sh0, s_l1, s_l2, s_l3)):
                    sched(t0 + 2 + 2 * i, fn)
                return

            # last group: run the tree on pipelined free-dim halves so the
            # post-loop drain chain is ~half as long (the shift DMAs of one
            # half overlap the other half's TT mins; two issue queues).
            def s_split():
                sh = sm_pool.tile([64, 8 * CHUNK], BF16, tag="sh")
                HF = 4 * CHUNK
                half = (slice(0, HF), slice(HF, 8 * CHUNK))
                qs = (nc.gpsimd, nc.scalar)
                for hh in range(2):
                    qs[hh].dma_start(sh[0:64, half[hh]],
                                     m4w_t[64:128, half[hh]])
                for w_ in (64, 32, 16):
                    for hh in range(2):
                        nc.vector.tensor_tensor(
                            out=m4w_t[0:w_, half[hh]],
                            in0=m4w_t[0:w_, half[hh]],
                            in1=sh[0:w_, half[hh]], op=MIN)
                        if w_ > 16:
                            qs[hh].dma_start(
                                sh[0:w_ // 2, half[hh]],
                                m4w_t[w_ // 2:w_, half[hh]])
                # coll split by co (contiguous output partitions)
                for hh in range(2):
                    qs[hh].dma_start(
                        collall[64 * hh:64 * hh + 64,
                                g * CHUNK:(g + 1) * CHUNK],
                        m4w_t[8 * hh:8 * hh + 8, :].rearrange(
                            "co (j s) -> co j s", j=8))
            sched(t0 + 2, s_split)

        # ---- phase 1: conv + depth-min --------------------------------
        items = [(n, hh) for n in range(NB) for hh in range(2)]
        quads = {}

        def prep_quad(k):
            n_k, hh_k = items[k]
            h0_k = 64 * hh_k
            hrows = 66 if hh_k == 0 else 64
            quad_k = in_pool.tile([128, QF], BF16, tag="quad")
            nc.gpsimd.memset(quad_k[:, hrows * W:QF], 0.0)
            # row 120 <- 1.0 from a DRAM constant via Sync DMA (a GpSimd
            # memset here would head-of-line block the tree-shift DMAs);
            # rows 121..127 are never streamed by any matmul.
            nc.sync.dma_start(quad_k[120:121, 0:hrows * W],
                              row1_ap[:, 0:hrows * W])
            qeng = (nc.sync, nc.gpsimd, nc.scalar, nc.sync)
            for r in range(4):
                nd = _strip_depths(r)
                srcr = x_ap[n_k, 8 * r:8 * r + nd, :,
                            h0_k:h0_k + hrows, :].rearrange(
                    "d c h w -> (d c) (h w)")
                # spread issue over queues so the 4 transfers overlap and
                # the first chunk's matmuls start sooner
                eng = qeng[r] if k == 0 else nc.sync
                eng.dma_start(
                    quad_k[32 * r:32 * r + 3 * nd, 0:hrows * W], srcr)
            quads[k] = quad_k

        prep_quad(0)
        load_consts()
        tick = 0
        cur = {}
        for k, (n, hh) in enumerate(items):
            quad = quads.pop(k)
            for q in range(2):
                m4w = m4w_pool.tile([128, 8 * CHUNK], BF16, tag="m4w")
                for j in range(8):
                    s0 = CHUNK * (8 * q + j)
                    ps = psum_pool.tile([128, 4 * CHUNK], F32, tag="big")
                    for khw in range(9):
                        kh, kw = khw // 3, khw % 3
                        koff = kh * W + kw
                        for r in range(4):
                            kr = _kr(r, khw)
                            nc.tensor.matmul(
                                ps[:, r * CHUNK:(r + 1) * CHUNK],
                                lhsT=w_sb[32 * r:32 * r + kr,
                                          khw * 128:(khw + 1) * 128],
                                rhs=quad[32 * r:32 * r + kr,
                                         s0 + koff:s0 + koff + CHUNK],
                                start=(khw == 0),
                                stop=(khw == 8),
                                tile_position=(32 * r, 0),
                                skip_group_check=True,
                            )
                    # ACT evacuates psum (f32 -> bf16); DVE folds the 4
                    # banks with two bf16 mins, merged across chunk pairs.
                    if j % 2 == 0:
                        c4 = ev_pool.tile([128, 8 * CHUNK], BF16, tag="c4")
                        cur["c4"] = c4
                    c4 = cur["c4"]
                    half = (j % 2) * 4 * CHUNK
                    nc.scalar.copy(c4[:, half:half + 4 * CHUNK], ps[:, :])
                    if j % 2 == 1:
                        h2 = ev_pool.tile([128, 4 * CHUNK], BF16, tag="h2")
                        c4v = c4.rearrange("p (c x) -> p c x", c=2)
                        nc.vector.tensor_tensor(
                            out=h2.rearrange("p (c x) -> p c x", c=2),
                            in0=c4v[:, :, 0:2 * CHUNK],
                            in1=c4v[:, :, 2 * CHUNK:4 * CHUNK], op=MIN)
                        h2v = h2.rearrange("p (c x) -> p c x", c=2)
                        nc.vector.tensor_tensor(
                            out=m4w[:, (j - 1) * CHUNK:(j + 1) * CHUNK]
                            .rearrange("p (c x) -> p c x", c=2),
                            in0=h2v[:, :, 0:CHUNK],
                            in1=h2v[:, :, CHUNK:2 * CHUNK], op=MIN)
                    run_tick(tick)
                    if q == 0 and j == 1 and k + 1 < len(items):
                        prep_quad(k + 1)
                    tick += 1
                g = 2 * k + q
                sched_group_tree(tick - 1, {"m4w": m4w, "g": g})
        for t in range(tick, tick + 10):
            run_tick(t)
        assert not actions, sorted(actions)

        # ---- phase 2: softmax over co, two pipelined 4-group halves ---
        # tile_wait_until pins these after the conv loop in the Tile
        # scheduler's model, so the FIFO ACT/DVE queues never carry a
        # phase-2 instruction in the middle of the psum-evacuation stream.
        ctx.enter_context(tc.tile_wait_until(100))
        HG = 4 * CHUNK
        for hhalf in range(2):
            gbase = 4 * hhalf
            csl = collall[:, gbase * CHUNK:gbase * CHUNK + HG]
            expt = sm_pool.tile([128, HG], BF16, tag="p2exp")
            nc.scalar.activation(
                expt[:, :], csl,
                mybir.ActivationFunctionType.Exp,
                bias=bias_sb[:, :], scale=1.0)
            tsum = psum_pool.tile([128, 4 * CHUNK], F32, tag="big")
            for gg in range(4):
                nc.tensor.matmul(
                    tsum[0:8, gg * CHUNK:(gg + 1) * CHUNK],
                    lhsT=ones_sb[:, :],
                    rhs=expt[:, gg * CHUNK:(gg + 1) * CHUNK],
                    start=True, stop=True)
            rec = sm_pool.tile([8, HG], F32, tag="p2rec")
            nc.vector.reciprocal_approx_fast(rec[:, :], tsum[0:8, :])
            tbc = psum_pool.tile([128, 4 * CHUNK], F32, tag="big")
            for gg in range(4):
                nc.tensor.matmul(
                    tbc[:, gg * CHUNK:(gg + 1) * CHUNK],
                    lhsT=ones_bc_sb[:, :],
                    rhs=rec[:, gg * CHUNK:(gg + 1) * CHUNK],
                    start=True, stop=True)
            soft = sm_pool.tile([128, HG], F32, tag="p2soft")
            nc.vector.tensor_tensor(
                out=soft[:, :], in0=tbc[:, :], in1=expt[:, :],
                op=mybir.AluOpType.mult)
            for gg in range(4):
                g = gbase + gg
                k, q = g // 2, g % 2
                n_g, hh_g = items[k]
                hc = 64 * hh_g + 32 * q
                # walk (co, h=(j,i), w) == src walk (p=8co+j, s=(i,w))
                dst = y_ap[n_g, :, hc:hc + 32, :].rearrange(
                    "co h w -> co (h w)")
                nc.sync.dma_start(dst,
                                  soft[:, gg * CHUNK:(gg + 1) * CHUNK])


def _compile():
    if "nc" in _COMPILED:
        return _COMPILED["nc"]
    nc = bacc.Bacc("TRN2", target_bir_lowering=False, debug=False,
                   num_devices=N_CORES)
    with tile.TileContext(nc) as tc:
        _emit_kernel(tc)
    nc.compile()
    _COMPILED["nc"] = nc
    return nc


def kernel(x, conv_weight, conv_bias):
    x = np.asarray(x, dtype=np.float32)
    conv_weight = np.asarray(conv_weight, dtype=np.float32)
    conv_bias = np.asarray(conv_bias, dtype=np.float32)

    xp = np.ascontiguousarray(
        x.transpose(0, 2, 1, 3, 4)).astype(ml_dtypes.bfloat16)  # [N,D,C,H,W]
    w_sb = _build_weight_blocks(conv_weight)
    bias_sb = _build_bias128(conv_bias)
    ones_sb = _build_ones()
    ones_bc_sb = _build_ones_bc()
    row1 = np.ones((1, 66 * W), dtype=ml_dtypes.bfloat16)

    nc = _compile()
    in_maps = []
    for i in range(N_CORES):
        in_maps.append({
            "x": np.ascontiguousarray(xp[NB * i:NB * (i + 1)]),
            "w": w_sb,
            "bias": bias_sb,
            "ones": ones_sb,
            "ones_bc": ones_bc_sb,
            "row1": row1,
        })
    res = bass_utils.run_bass_kernel_spmd(
        nc, in_maps, core_ids=list(range(N_CORES)),
        trace=bool(int(os.environ.get("KERNEL_TRACE", "0"))),
    )
    _COMPILED["last_results"] = res
    out = np.concatenate(
        [res.results[i]["y"][:, :, :HOUT, :WOUT] for i in range(N_CORES)],
        axis=0)
    return out


if __name__ == "__main__":
    _compile()
    print("build OK")
